# revision 3
# baseline (speedup 1.0000x reference)
"""Trainium2 Bass kernel for nn_ContrastiveLoss (retrieval_knn).

Changes vs baseline kernel.py:
- all DMA on hardware-DGE queues (sync + scalar), none on gpsimd/SWDGE;
  rhs rows alternate between the two queues.
- consolidated matmuls: one matmul per (half, chunk, slab) -> 16 qc matmuls
  per pair instead of 56 (fewer LDWEIGHTS + pipeline drains).
- c2 folded via a K=6 fp8 selector matmul (hi/mid/lo planes) instead of the
  f32r K=4 matmul: same N but fp8 streams 2 cols/cycle.
"""

import numpy as np
import ml_dtypes

B = 222
NB = 444
T = 64
D = 512
V = 6
K = 54
NPOS = 2 * V
EPS = 1e-8
LN_EPS = float(np.log(np.float32(1e-8)))
NCORES = 8
BL = 28
PAIRS = BL // 2
CH = 4
SLABS = [(0, 32), (32, 22)]   # candidate slabs: 2048 + 1408 cols
SLAB_BLOCKS = {
    0: [(0, 512), (512, 512), (1024, 512), (1536, 512)],
    1: [(0, 512), (512, 512), (1024, 384)],
}
C0 = 512.0

CORE_STARTS = [0, 28, 56, 84, 112, 140, 168, 195]
CORE_COUNTS = [28, 28, 28, 28, 28, 28, 27, 27]

LAST_EXEC_NS = None
LAST_RESULTS = None


def _fp8_triple(x):
    hi = x.astype(ml_dtypes.float8_e4m3fn)
    r1 = x - hi.astype(np.float32)
    mid = r1.astype(ml_dtypes.float8_e4m3fn)
    lo = (r1 - mid.astype(np.float32)).astype(ml_dtypes.float8_e4m3fn)
    return hi, mid, lo


def _prep(inputs):
    emb = np.ascontiguousarray(np.asarray(inputs["embeddings"]), dtype=np.float32)
    ips = np.asarray(inputs["indices_posself"]).astype(np.int64)
    ipc = np.asarray(inputs["indices_poscross"]).astype(np.int64)
    ineg = np.asarray(inputs["indices_neg"]).astype(np.int64)
    osh = np.asarray(inputs["order_to_shuffle"]).astype(np.int64)
    pos = np.concatenate([ips, ipc], axis=1)
    combined = np.concatenate([pos, ineg, osh[pos]], axis=1)  # (222, 54)

    bank8 = emb.astype(ml_dtypes.float8_e4m3fn)
    bankf = bank8.astype(np.float32)
    c2 = np.einsum(
        "jsd,jsd->js", bankf.astype(np.float64), bankf.astype(np.float64)
    ).astype(np.float32)

    lhs_all = (-2.0 * bankf[:B]).astype(ml_dtypes.float8_e4m3fn)

    sel = np.zeros((6, 128), np.float32)
    sel[0:3, 0:64] = 1.0
    sel[3:6, 64:128] = 1.0
    sel8 = sel.astype(ml_dtypes.float8_e4m3fn)

    in_maps = []
    for ci in range(NCORES):
        s, n = CORE_STARTS[ci], CORE_COUNTS[ci]
        rows = np.array(list(range(s, s + n)) + [s] * (BL - n))
        cmb = combined[rows]

        g = bank8[cmb]                                        # (28,54,64,512) fp8
        rhs = np.ascontiguousarray(
            g.reshape(BL, K, T, CH, 128).transpose(0, 4, 3, 1, 2).reshape(BL, 128, CH, K * T)
        )
        lt = np.ascontiguousarray(
            lhs_all[rows].reshape(BL, T, CH, 128).transpose(0, 3, 2, 1)
        )                                                     # (28,128,4,64)

        c2g = c2[cmb].reshape(BL, K * T) - C0
        hi, mid, lo = _fp8_triple(c2g)
        c2p = np.empty((PAIRS, 6, K * T), ml_dtypes.float8_e4m3fn)
        c2p[:, 0] = hi[0::2]
        c2p[:, 1] = mid[0::2]
        c2p[:, 2] = lo[0::2]
        c2p[:, 3] = hi[1::2]
        c2p[:, 4] = mid[1::2]
        c2p[:, 5] = lo[1::2]

        q2row = c2[rows]
        q2n = np.empty((128, PAIRS), np.float32)
        for p in range(PAIRS):
            q2n[0:64, p] = -(q2row[2 * p] + C0)
            q2n[64:128, p] = -(q2row[2 * p + 1] + C0)
        q2h = (q2n - LN_EPS).astype(np.float32)
        in_maps.append(
            {"rhs": rhs, "lhsT": lt, "c2p": c2p, "q2n": q2n, "q2h": q2h, "sel": sel8}
        )
    return in_maps


def _build(nc):
    import concourse.tile as tile
    import concourse.mybir as mybir
    from contextlib import ExitStack

    dt = mybir.dt
    f32 = dt.float32
    fp8 = dt.float8e4

    rhs_d = nc.dram_tensor("rhs", [BL, 128, CH, K * T], fp8, kind="ExternalInput")
    lhsT_d = nc.dram_tensor("lhsT", [BL, 128, CH, T], fp8, kind="ExternalInput")
    c2p_d = nc.dram_tensor("c2p", [PAIRS, 6, K * T], fp8, kind="ExternalInput")
    q2_d = nc.dram_tensor("q2n", [128, PAIRS], f32, kind="ExternalInput")
    q2h_d = nc.dram_tensor("q2h", [128, PAIRS], f32, kind="ExternalInput")
    sel_d = nc.dram_tensor("sel", [6, 128], fp8, kind="ExternalInput")
    out_d = nc.dram_tensor("out", [128, PAIRS], f32, kind="ExternalOutput")

    with tile.TileContext(nc) as tc, ExitStack() as ctx:
        rhs_pool = ctx.enter_context(tc.tile_pool(name="rhs", bufs=12))
        lhs_pool = ctx.enter_context(tc.tile_pool(name="lhs", bufs=6))
        c2_pool = ctx.enter_context(tc.tile_pool(name="c2", bufs=3))
        ps_pool = ctx.enter_context(tc.tile_pool(name="ps", bufs=2, space="PSUM"))
        m_pool = ctx.enter_context(tc.tile_pool(name="m", bufs=3))
        e_pool = ctx.enter_context(tc.tile_pool(name="e", bufs=3))
        s_pool = ctx.enter_context(tc.tile_pool(name="s", bufs=1))

        sel = s_pool.tile([6, 128], fp8)
        nc.scalar.dma_start(sel[:], sel_d[:])
        q2t = s_pool.tile([128, PAIRS], f32)
        nc.scalar.dma_start(q2t[:], q2_d[:])
        q2ht = s_pool.tile([128, PAIRS], f32)
        nc.scalar.dma_start(q2ht[:], q2h_d[:])
        possum = s_pool.tile([128, PAIRS], f32)
        negsum = s_pool.tile([128, PAIRS], f32)

        # rhs rows round-robin across the two HWDGE queues (sync, scalar)
        # and the SWDGE queue (gpsimd); small per-pair tensors ride scalar.
        rhs_q = [nc.sync, nc.scalar, nc.gpsimd]
        for p in range(PAIRS):
            b0, b1 = 2 * p, 2 * p + 1
            lt0 = lhs_pool.tile([128, CH, T], fp8, tag="lhs")
            nc.scalar.dma_start(lt0[:], lhsT_d[b0])
            lt1 = lhs_pool.tile([128, CH, T], fp8, tag="lhs")
            nc.scalar.dma_start(lt1[:], lhsT_d[b1])
            c2t = c2_pool.tile([6, K * T], fp8, tag="c2")
            nc.scalar.dma_start(c2t[:], c2p_d[p])

            m = m_pool.tile([128, K], f32, tag="m")

            # each row's rhs arrives as two chunk-half transfers on different
            # queues: halves the per-pair arrival latency and keeps all three
            # DMA queues temporally aligned on the same pair window.
            r0 = rhs_pool.tile([128, CH, K * T], fp8, tag="rhs")
            rhs_q[(2 * b0) % 3].dma_start(r0[:, 0:2], rhs_d[b0, :, 0:2])
            rhs_q[(2 * b0 + 1) % 3].dma_start(r0[:, 2:4], rhs_d[b0, :, 2:4])
            r1 = rhs_pool.tile([128, CH, K * T], fp8, tag="rhs")
            rhs_q[(2 * b1) % 3].dma_start(r1[:, 0:2], rhs_d[b1, :, 0:2])
            rhs_q[(2 * b1 + 1) % 3].dma_start(r1[:, 2:4], rhs_d[b1, :, 2:4])

            for h, (k0, kw) in enumerate(SLABS):
                w = kw * T
                ps = ps_pool.tile([128, w], f32, tag="ps")
                for c in range(CH):
                    for off, nw in SLAB_BLOCKS[h]:
                        nc.tensor.matmul(
                            ps[0:64, off:off + nw],
                            lt0[:, c, :],
                            r0[:, c, k0 * T + off:k0 * T + off + nw],
                            start=(c == 0), stop=False,
                            tile_position=(0, 0),
                        )
                        nc.tensor.matmul(
                            ps[64:128, off:off + nw],
                            lt1[:, c, :],
                            r1[:, c, k0 * T + off:k0 * T + off + nw],
                            start=(c == 0), stop=False,
                            tile_position=(0, 64),
                        )
                for off, nw in SLAB_BLOCKS[h]:
                    nc.tensor.matmul(
                        ps[:, off:off + nw],
                        sel[:],
                        c2t[:, k0 * T + off:k0 * T + off + nw],
                        start=False, stop=True,
                    )
                nc.vector.tensor_reduce(
                    out=m[:, k0:k0 + kw],
                    in_=ps[:].rearrange("p (k s) -> p k s", s=T),
                    op=mybir.AluOpType.min,
                    axis=mybir.AxisListType.X,
                )

            # clamp m into [q2n, q2n - ln eps] (per partition) so that
            # exp(-mc + q2n) == clamp(exp(-m + q2n), eps, 1) exactly; then the
            # scalar engine produces pos/neg sums via its accumulator.
            mc = e_pool.tile([128, K], f32, tag="e")
            nc.vector.tensor_scalar(
                out=mc[:],
                in0=m[:],
                scalar1=q2t[:, p:p + 1],
                scalar2=q2ht[:, p:p + 1],
                op0=mybir.AluOpType.max,
                op1=mybir.AluOpType.min,
            )
            ed = e_pool.tile([128, K], f32, tag="ed")
            nc.scalar.activation(
                ed[:, 0:NPOS],
                mc[:, 0:NPOS],
                mybir.ActivationFunctionType.Exp,
                bias=q2t[:, p:p + 1],
                scale=-1.0,
                accum_out=possum[:, p:p + 1],
            )
            nc.scalar.activation(
                ed[:, NPOS:K],
                mc[:, NPOS:K],
                mybir.ActivationFunctionType.Exp,
                bias=q2t[:, p:p + 1],
                scale=-1.0,
                accum_out=negsum[:, p:p + 1],
            )

        den = s_pool.tile([128, PAIRS], f32)
        nc.vector.tensor_add(den[:], possum[:], negsum[:])
        nc.vector.tensor_scalar_add(den[:], den[:], EPS)
        nc.vector.reciprocal(den[:], den[:])
        nc.vector.tensor_mul(den[:], den[:], possum[:])
        lnr = s_pool.tile([128, PAIRS], f32)
        nc.scalar.activation(lnr[:], den[:], mybir.ActivationFunctionType.Ln)
        nc.sync.dma_start(out_d[:], lnr[:])


def _ensure_axon_hooks():
    """bass_utils' trace path imports antenv.axon_hooks, which this image
    lacks; install a functional shim driving NTFF capture via libaxon."""
    try:
        import antenv.axon_hooks  # noqa: F401

        return
    except ImportError:
        pass
    import contextlib
    import ctypes
    import os
    import sys
    import types

    try:
        import antenv
    except ImportError:
        return
    mod = types.ModuleType("antenv.axon_hooks")
    _hook_box = [None]
    mod.set_axon_ntff_profile_hook = lambda h: _hook_box.__setitem__(0, h)
    mod.get_axon_ntff_profile_hook = lambda: _hook_box[0]
    sys.modules["antenv.axon_hooks"] = mod
    antenv.axon_hooks = mod

    so_path = "/opt/axon/libaxon_pjrt.so"
    if not os.path.exists(so_path):
        return
    try:
        lib = ctypes.CDLL(so_path)
        if not hasattr(lib, "axon_start_nrt_profile"):
            return
        lib.axon_start_nrt_profile.argtypes = [
            ctypes.POINTER(ctypes.c_int64),
            ctypes.c_size_t,
        ]
        lib.axon_start_nrt_profile.restype = ctypes.c_int64
        lib.axon_stop_nrt_profile.argtypes = [ctypes.c_char_p]
        lib.axon_stop_nrt_profile.restype = ctypes.c_int64

        @contextlib.contextmanager
        def _hook(output_dir, device_ids):
            import jax

            jax.devices()
            if device_ids:
                ids = (ctypes.c_int64 * len(device_ids))(*device_ids)
                rc = lib.axon_start_nrt_profile(ids, len(device_ids))
            else:
                rc = lib.axon_start_nrt_profile(None, 0)
            if rc != 0:
                raise RuntimeError(f"axon_start_nrt_profile rc={rc}")
            try:
                yield
            finally:
                n = lib.axon_stop_nrt_profile(str(output_dir).encode())
                print(f"profile: {n} file(s) written to {output_dir}", file=sys.stderr)

        mod.set_axon_ntff_profile_hook(_hook)
    except Exception:
        pass


def kernel(**inputs):
    global LAST_EXEC_NS, LAST_RESULTS
    import sys
    import time

    _ensure_axon_hooks()
    import concourse.bacc as bacc
    from concourse.bass_utils import run_bass_kernel_spmd

    t0 = time.time()
    in_maps = _prep(inputs)
    print(f"[v2] prep done {time.time()-t0:.1f}s", file=sys.stderr, flush=True)
    nc = bacc.Bacc("TRN2", target_bir_lowering=False, debug=False, num_devices=NCORES)
    _build(nc)
    nc.finalize()
    print(f"[v2] build done {time.time()-t0:.1f}s", file=sys.stderr, flush=True)
    res = run_bass_kernel_spmd(nc, in_maps, list(range(NCORES)))
    print(f"[v2] run done {time.time()-t0:.1f}s", file=sys.stderr, flush=True)
    LAST_EXEC_NS = res.exec_time_ns
    LAST_RESULTS = res

    total = 0.0
    for ci in range(NCORES):
        lnr = np.asarray(res.results[ci]["out"], dtype=np.float64)
        n = CORE_COUNTS[ci]
        for bl in range(n):
            pr, half = bl // 2, bl % 2
            total += lnr[half * 64:(half + 1) * 64, pr].sum()
    return np.float32(-500.0 * total / float(B))


# revision 4
# speedup vs baseline: 1.0567x; 1.0567x over previous
"""Trainium2 Bass kernel for nn_ContrastiveLoss (retrieval_knn).

Changes vs baseline kernel.py:
- all DMA on hardware-DGE queues (sync + scalar), none on gpsimd/SWDGE;
  rhs rows alternate between the two queues.
- consolidated matmuls: one matmul per (half, chunk, slab) -> 16 qc matmuls
  per pair instead of 56 (fewer LDWEIGHTS + pipeline drains).
- c2 folded via a K=6 fp8 selector matmul (hi/mid/lo planes) instead of the
  f32r K=4 matmul: same N but fp8 streams 2 cols/cycle.
"""

import numpy as np
import ml_dtypes

B = 222
NB = 444
T = 64
D = 512
V = 6
K = 54
NPOS = 2 * V
EPS = 1e-8
LN_EPS = float(np.log(np.float32(1e-8)))
NCORES = 8
BL = 28
PAIRS = BL // 2
CH = 4
SLABS = [(0, 32), (32, 22)]   # candidate slabs: 2048 + 1408 cols
SLAB_BLOCKS = {
    0: [(0, 512), (512, 512), (1024, 512), (1536, 512)],
    1: [(0, 512), (512, 512), (1024, 384)],
}
C0 = 512.0

CORE_STARTS = [0, 28, 56, 84, 112, 140, 168, 195]
CORE_COUNTS = [28, 28, 28, 28, 28, 28, 27, 27]

LAST_EXEC_NS = None
LAST_RESULTS = None


def _fp8_triple(x):
    hi = x.astype(ml_dtypes.float8_e4m3fn)
    r1 = x - hi.astype(np.float32)
    mid = r1.astype(ml_dtypes.float8_e4m3fn)
    lo = (r1 - mid.astype(np.float32)).astype(ml_dtypes.float8_e4m3fn)
    return hi, mid, lo


def _prep(inputs):
    emb = np.ascontiguousarray(np.asarray(inputs["embeddings"]), dtype=np.float32)
    ips = np.asarray(inputs["indices_posself"]).astype(np.int64)
    ipc = np.asarray(inputs["indices_poscross"]).astype(np.int64)
    ineg = np.asarray(inputs["indices_neg"]).astype(np.int64)
    osh = np.asarray(inputs["order_to_shuffle"]).astype(np.int64)
    pos = np.concatenate([ips, ipc], axis=1)
    combined = np.concatenate([pos, ineg, osh[pos]], axis=1)  # (222, 54)

    bank8 = emb.astype(ml_dtypes.float8_e4m3fn)
    bankf = bank8.astype(np.float32)
    c2 = np.einsum(
        "jsd,jsd->js", bankf.astype(np.float64), bankf.astype(np.float64)
    ).astype(np.float32)

    lhs_all = (-2.0 * bankf[:B]).astype(ml_dtypes.float8_e4m3fn)

    sel = np.zeros((6, 128), np.float32)
    sel[0:3, 0:64] = 1.0
    sel[3:6, 64:128] = 1.0
    sel8 = sel.astype(ml_dtypes.float8_e4m3fn)

    in_maps = []
    for ci in range(NCORES):
        s, n = CORE_STARTS[ci], CORE_COUNTS[ci]
        rows = np.array(list(range(s, s + n)) + [s] * (BL - n))
        cmb = combined[rows]

        g = bank8[cmb]                                        # (28,54,64,512) fp8
        rhs = np.ascontiguousarray(
            g.reshape(BL, K, T, CH, 128).transpose(0, 4, 3, 1, 2).reshape(BL, 128, CH, K * T)
        )
        lt = np.ascontiguousarray(
            lhs_all[rows].reshape(BL, T, CH, 128).transpose(0, 3, 2, 1)
        )                                                     # (28,128,4,64)

        c2g = c2[cmb].reshape(BL, K * T) - C0
        hi, mid, lo = _fp8_triple(c2g)
        c2p = np.empty((PAIRS, 6, K * T), ml_dtypes.float8_e4m3fn)
        c2p[:, 0] = hi[0::2]
        c2p[:, 1] = mid[0::2]
        c2p[:, 2] = lo[0::2]
        c2p[:, 3] = hi[1::2]
        c2p[:, 4] = mid[1::2]
        c2p[:, 5] = lo[1::2]

        q2row = c2[rows]
        q2n = np.empty((128, PAIRS), np.float32)
        for p in range(PAIRS):
            q2n[0:64, p] = -(q2row[2 * p] + C0)
            q2n[64:128, p] = -(q2row[2 * p + 1] + C0)
        q2h = (q2n - LN_EPS).astype(np.float32)
        in_maps.append(
            {"rhs": rhs, "lhsT": lt, "c2p": c2p, "q2n": q2n, "q2h": q2h, "sel": sel8}
        )
    return in_maps


def _build(nc):
    import concourse.tile as tile
    import concourse.mybir as mybir
    from contextlib import ExitStack

    dt = mybir.dt
    f32 = dt.float32
    fp8 = dt.float8e4

    rhs_d = nc.dram_tensor("rhs", [BL, 128, CH, K * T], fp8, kind="ExternalInput")
    lhsT_d = nc.dram_tensor("lhsT", [BL, 128, CH, T], fp8, kind="ExternalInput")
    c2p_d = nc.dram_tensor("c2p", [PAIRS, 6, K * T], fp8, kind="ExternalInput")
    q2_d = nc.dram_tensor("q2n", [128, PAIRS], f32, kind="ExternalInput")
    q2h_d = nc.dram_tensor("q2h", [128, PAIRS], f32, kind="ExternalInput")
    sel_d = nc.dram_tensor("sel", [6, 128], fp8, kind="ExternalInput")
    out_d = nc.dram_tensor("out", [128, PAIRS], f32, kind="ExternalOutput")

    with tile.TileContext(nc) as tc, ExitStack() as ctx:
        rhs_pool = ctx.enter_context(tc.tile_pool(name="rhs", bufs=12))
        lhs_pool = ctx.enter_context(tc.tile_pool(name="lhs", bufs=6))
        c2_pool = ctx.enter_context(tc.tile_pool(name="c2", bufs=3))
        ps_pool = ctx.enter_context(tc.tile_pool(name="ps", bufs=2, space="PSUM"))
        m_pool = ctx.enter_context(tc.tile_pool(name="m", bufs=3))
        e_pool = ctx.enter_context(tc.tile_pool(name="e", bufs=3))
        s_pool = ctx.enter_context(tc.tile_pool(name="s", bufs=1))

        sel = s_pool.tile([6, 128], fp8)
        nc.scalar.dma_start(sel[:], sel_d[:])
        q2t = s_pool.tile([128, PAIRS], f32)
        nc.scalar.dma_start(q2t[:], q2_d[:])
        q2ht = s_pool.tile([128, PAIRS], f32)
        nc.scalar.dma_start(q2ht[:], q2h_d[:])
        possum = s_pool.tile([128, PAIRS], f32)
        negsum = s_pool.tile([128, PAIRS], f32)

        # rhs rows round-robin across the two HWDGE queues (sync, scalar)
        # and the SWDGE queue (gpsimd); small per-pair tensors ride scalar.
        rhs_q = [nc.sync, nc.scalar, nc.gpsimd]

        # transfer issue is software-pipelined two pairs ahead of compute so
        # the scalar engine's exp work never gates posting of upcoming
        # descriptors to its DMA queue.
        tiles = {}

        def issue(p):
            b0, b1 = 2 * p, 2 * p + 1
            # each row's rhs arrives as two chunk-half transfers on different
            # queues: halves the per-pair arrival latency and keeps all three
            # DMA queues temporally aligned on the same pair window.
            r0 = rhs_pool.tile([128, CH, K * T], fp8, tag="rhs")
            rhs_q[(2 * b0) % 3].dma_start(r0[:, 0:2], rhs_d[b0, :, 0:2])
            rhs_q[(2 * b0 + 1) % 3].dma_start(r0[:, 2:4], rhs_d[b0, :, 2:4])
            r1 = rhs_pool.tile([128, CH, K * T], fp8, tag="rhs")
            rhs_q[(2 * b1) % 3].dma_start(r1[:, 0:2], rhs_d[b1, :, 0:2])
            rhs_q[(2 * b1 + 1) % 3].dma_start(r1[:, 2:4], rhs_d[b1, :, 2:4])
            lt0 = lhs_pool.tile([128, CH, T], fp8, tag="lhs")
            nc.scalar.dma_start(lt0[:], lhsT_d[b0])
            lt1 = lhs_pool.tile([128, CH, T], fp8, tag="lhs")
            nc.scalar.dma_start(lt1[:], lhsT_d[b1])
            c2t = c2_pool.tile([6, K * T], fp8, tag="c2")
            nc.scalar.dma_start(c2t[:], c2p_d[p])
            tiles[p] = (r0, r1, lt0, lt1, c2t)

        issue(0)
        issue(1)
        for p in range(PAIRS):
            if p + 2 < PAIRS:
                issue(p + 2)
            b0, b1 = 2 * p, 2 * p + 1
            r0, r1, lt0, lt1, c2t = tiles.pop(p)

            m = m_pool.tile([128, K], f32, tag="m")

            for h, (k0, kw) in enumerate(SLABS):
                w = kw * T
                ps = ps_pool.tile([128, w], f32, tag="ps")
                for c in range(CH):
                    for off, nw in SLAB_BLOCKS[h]:
                        nc.tensor.matmul(
                            ps[0:64, off:off + nw],
                            lt0[:, c, :],
                            r0[:, c, k0 * T + off:k0 * T + off + nw],
                            start=(c == 0), stop=False,
                            tile_position=(0, 0),
                        )
                        nc.tensor.matmul(
                            ps[64:128, off:off + nw],
                            lt1[:, c, :],
                            r1[:, c, k0 * T + off:k0 * T + off + nw],
                            start=(c == 0), stop=False,
                            tile_position=(0, 64),
                        )
                for off, nw in SLAB_BLOCKS[h]:
                    nc.tensor.matmul(
                        ps[:, off:off + nw],
                        sel[:],
                        c2t[:, k0 * T + off:k0 * T + off + nw],
                        start=False, stop=True,
                    )
                nc.vector.tensor_reduce(
                    out=m[:, k0:k0 + kw],
                    in_=ps[:].rearrange("p (k s) -> p k s", s=T),
                    op=mybir.AluOpType.min,
                    axis=mybir.AxisListType.X,
                )

            # clamp m into [q2n, q2n - ln eps] (per partition) so that
            # exp(-mc + q2n) == clamp(exp(-m + q2n), eps, 1) exactly; then the
            # scalar engine produces pos/neg sums via its accumulator.
            mc = e_pool.tile([128, K], f32, tag="e")
            nc.vector.tensor_scalar(
                out=mc[:],
                in0=m[:],
                scalar1=q2t[:, p:p + 1],
                scalar2=q2ht[:, p:p + 1],
                op0=mybir.AluOpType.max,
                op1=mybir.AluOpType.min,
            )
            ed = e_pool.tile([128, K], f32, tag="ed")
            nc.scalar.activation(
                ed[:, 0:NPOS],
                mc[:, 0:NPOS],
                mybir.ActivationFunctionType.Exp,
                bias=q2t[:, p:p + 1],
                scale=-1.0,
                accum_out=possum[:, p:p + 1],
            )
            nc.scalar.activation(
                ed[:, NPOS:K],
                mc[:, NPOS:K],
                mybir.ActivationFunctionType.Exp,
                bias=q2t[:, p:p + 1],
                scale=-1.0,
                accum_out=negsum[:, p:p + 1],
            )

        den = s_pool.tile([128, PAIRS], f32)
        nc.vector.tensor_add(den[:], possum[:], negsum[:])
        nc.vector.tensor_scalar_add(den[:], den[:], EPS)
        nc.vector.reciprocal(den[:], den[:])
        nc.vector.tensor_mul(den[:], den[:], possum[:])
        lnr = s_pool.tile([128, PAIRS], f32)
        nc.scalar.activation(lnr[:], den[:], mybir.ActivationFunctionType.Ln)
        nc.sync.dma_start(out_d[:], lnr[:])


def _ensure_axon_hooks():
    """bass_utils' trace path imports antenv.axon_hooks, which this image
    lacks; install a functional shim driving NTFF capture via libaxon."""
    try:
        import antenv.axon_hooks  # noqa: F401

        return
    except ImportError:
        pass
    import contextlib
    import ctypes
    import os
    import sys
    import types

    try:
        import antenv
    except ImportError:
        return
    mod = types.ModuleType("antenv.axon_hooks")
    _hook_box = [None]
    mod.set_axon_ntff_profile_hook = lambda h: _hook_box.__setitem__(0, h)
    mod.get_axon_ntff_profile_hook = lambda: _hook_box[0]
    sys.modules["antenv.axon_hooks"] = mod
    antenv.axon_hooks = mod

    so_path = "/opt/axon/libaxon_pjrt.so"
    if not os.path.exists(so_path):
        return
    try:
        lib = ctypes.CDLL(so_path)
        if not hasattr(lib, "axon_start_nrt_profile"):
            return
        lib.axon_start_nrt_profile.argtypes = [
            ctypes.POINTER(ctypes.c_int64),
            ctypes.c_size_t,
        ]
        lib.axon_start_nrt_profile.restype = ctypes.c_int64
        lib.axon_stop_nrt_profile.argtypes = [ctypes.c_char_p]
        lib.axon_stop_nrt_profile.restype = ctypes.c_int64

        @contextlib.contextmanager
        def _hook(output_dir, device_ids):
            import jax

            jax.devices()
            if device_ids:
                ids = (ctypes.c_int64 * len(device_ids))(*device_ids)
                rc = lib.axon_start_nrt_profile(ids, len(device_ids))
            else:
                rc = lib.axon_start_nrt_profile(None, 0)
            if rc != 0:
                raise RuntimeError(f"axon_start_nrt_profile rc={rc}")
            try:
                yield
            finally:
                n = lib.axon_stop_nrt_profile(str(output_dir).encode())
                print(f"profile: {n} file(s) written to {output_dir}", file=sys.stderr)

        mod.set_axon_ntff_profile_hook(_hook)
    except Exception:
        pass


def kernel(**inputs):
    global LAST_EXEC_NS, LAST_RESULTS
    import sys
    import time

    _ensure_axon_hooks()
    import concourse.bacc as bacc
    from concourse.bass_utils import run_bass_kernel_spmd

    t0 = time.time()
    in_maps = _prep(inputs)
    print(f"[v2] prep done {time.time()-t0:.1f}s", file=sys.stderr, flush=True)
    nc = bacc.Bacc("TRN2", target_bir_lowering=False, debug=False, num_devices=NCORES)
    _build(nc)
    nc.finalize()
    print(f"[v2] build done {time.time()-t0:.1f}s", file=sys.stderr, flush=True)
    res = run_bass_kernel_spmd(nc, in_maps, list(range(NCORES)))
    print(f"[v2] run done {time.time()-t0:.1f}s", file=sys.stderr, flush=True)
    LAST_EXEC_NS = res.exec_time_ns
    LAST_RESULTS = res

    total = 0.0
    for ci in range(NCORES):
        lnr = np.asarray(res.results[ci]["out"], dtype=np.float64)
        n = CORE_COUNTS[ci]
        for bl in range(n):
            pr, half = bl // 2, bl % 2
            total += lnr[half * 64:(half + 1) * 64, pr].sum()
    return np.float32(-500.0 * total / float(B))


# revision 5
# speedup vs baseline: 1.0808x; 1.0228x over previous
"""Trainium2 Bass kernel for nn_ContrastiveLoss (retrieval_knn).

Changes vs baseline kernel.py:
- all DMA on hardware-DGE queues (sync + scalar), none on gpsimd/SWDGE;
  rhs rows alternate between the two queues.
- consolidated matmuls: one matmul per (half, chunk, slab) -> 16 qc matmuls
  per pair instead of 56 (fewer LDWEIGHTS + pipeline drains).
- c2 folded via a K=6 fp8 selector matmul (hi/mid/lo planes) instead of the
  f32r K=4 matmul: same N but fp8 streams 2 cols/cycle.
"""

import numpy as np
import ml_dtypes

B = 222
NB = 444
T = 64
D = 512
V = 6
K = 54
NPOS = 2 * V
EPS = 1e-8
LN_EPS = float(np.log(np.float32(1e-8)))
NCORES = 8
BL = 28
PAIRS = BL // 2
CH = 4
SLABS = [(0, 32), (32, 22)]   # candidate slabs: 2048 + 1408 cols
SLAB_BLOCKS = {
    0: [(0, 512), (512, 512), (1024, 512), (1536, 512)],
    1: [(0, 512), (512, 512), (1024, 384)],
}
C0 = 512.0

CORE_STARTS = [0, 28, 56, 84, 112, 140, 168, 195]
CORE_COUNTS = [28, 28, 28, 28, 28, 28, 27, 27]

LAST_EXEC_NS = None
LAST_RESULTS = None


def _fp8_triple(x):
    hi = x.astype(ml_dtypes.float8_e4m3fn)
    r1 = x - hi.astype(np.float32)
    mid = r1.astype(ml_dtypes.float8_e4m3fn)
    lo = (r1 - mid.astype(np.float32)).astype(ml_dtypes.float8_e4m3fn)
    return hi, mid, lo


def _prep(inputs):
    emb = np.ascontiguousarray(np.asarray(inputs["embeddings"]), dtype=np.float32)
    ips = np.asarray(inputs["indices_posself"]).astype(np.int64)
    ipc = np.asarray(inputs["indices_poscross"]).astype(np.int64)
    ineg = np.asarray(inputs["indices_neg"]).astype(np.int64)
    osh = np.asarray(inputs["order_to_shuffle"]).astype(np.int64)
    pos = np.concatenate([ips, ipc], axis=1)
    combined = np.concatenate([pos, ineg, osh[pos]], axis=1)  # (222, 54)

    bank8 = emb.astype(ml_dtypes.float8_e4m3fn)
    bankf = bank8.astype(np.float32)
    c2 = np.einsum(
        "jsd,jsd->js", bankf.astype(np.float64), bankf.astype(np.float64)
    ).astype(np.float32)

    lhs_all = (-2.0 * bankf[:B]).astype(ml_dtypes.float8_e4m3fn)

    sel = np.zeros((6, 128), np.float32)
    sel[0:3, 0:64] = 1.0
    sel[3:6, 64:128] = 1.0
    sel8 = sel.astype(ml_dtypes.float8_e4m3fn)

    in_maps = []
    for ci in range(NCORES):
        s, n = CORE_STARTS[ci], CORE_COUNTS[ci]
        rows = np.array(list(range(s, s + n)) + [s] * (BL - n))
        cmb = combined[rows]

        g = bank8[cmb]                                        # (28,54,64,512) fp8
        rhs = np.ascontiguousarray(
            g.reshape(BL, K, T, CH, 128).transpose(0, 4, 3, 1, 2).reshape(BL, 128, CH, K * T)
        )
        lt = np.ascontiguousarray(
            lhs_all[rows].reshape(BL, T, CH, 128).transpose(0, 3, 2, 1)
        )                                                     # (28,128,4,64)

        c2g = c2[cmb].reshape(BL, K * T) - C0
        hi, mid, lo = _fp8_triple(c2g)
        c2p = np.empty((PAIRS, 6, K * T), ml_dtypes.float8_e4m3fn)
        c2p[:, 0] = hi[0::2]
        c2p[:, 1] = mid[0::2]
        c2p[:, 2] = lo[0::2]
        c2p[:, 3] = hi[1::2]
        c2p[:, 4] = mid[1::2]
        c2p[:, 5] = lo[1::2]

        q2row = c2[rows]
        q2n = np.empty((128, PAIRS), np.float32)
        for p in range(PAIRS):
            q2n[0:64, p] = -(q2row[2 * p] + C0)
            q2n[64:128, p] = -(q2row[2 * p + 1] + C0)
        q2h = (q2n - LN_EPS).astype(np.float32)
        in_maps.append(
            {"rhs": rhs, "lhsT": lt, "c2p": c2p, "q2n": q2n, "q2h": q2h, "sel": sel8}
        )
    return in_maps


def _build(nc):
    import concourse.tile as tile
    import concourse.mybir as mybir
    from contextlib import ExitStack

    dt = mybir.dt
    f32 = dt.float32
    fp8 = dt.float8e4

    rhs_d = nc.dram_tensor("rhs", [BL, 128, CH, K * T], fp8, kind="ExternalInput")
    lhsT_d = nc.dram_tensor("lhsT", [BL, 128, CH, T], fp8, kind="ExternalInput")
    c2p_d = nc.dram_tensor("c2p", [PAIRS, 6, K * T], fp8, kind="ExternalInput")
    q2_d = nc.dram_tensor("q2n", [128, PAIRS], f32, kind="ExternalInput")
    q2h_d = nc.dram_tensor("q2h", [128, PAIRS], f32, kind="ExternalInput")
    sel_d = nc.dram_tensor("sel", [6, 128], fp8, kind="ExternalInput")
    out_d = nc.dram_tensor("out", [128, PAIRS], f32, kind="ExternalOutput")

    with tile.TileContext(nc) as tc, ExitStack() as ctx:
        rhs_pool = ctx.enter_context(tc.tile_pool(name="rhs", bufs=12))
        lhs_pool = ctx.enter_context(tc.tile_pool(name="lhs", bufs=10))
        c2_pool = ctx.enter_context(tc.tile_pool(name="c2", bufs=5))
        ps_pool = ctx.enter_context(tc.tile_pool(name="ps", bufs=2, space="PSUM"))
        m_pool = ctx.enter_context(tc.tile_pool(name="m", bufs=3))
        e_pool = ctx.enter_context(tc.tile_pool(name="e", bufs=3))
        s_pool = ctx.enter_context(tc.tile_pool(name="s", bufs=1))

        sel = s_pool.tile([6, 128], fp8)
        nc.scalar.dma_start(sel[:], sel_d[:])
        q2t = s_pool.tile([128, PAIRS], f32)
        nc.scalar.dma_start(q2t[:], q2_d[:])
        q2ht = s_pool.tile([128, PAIRS], f32)
        nc.scalar.dma_start(q2ht[:], q2h_d[:])
        possum = s_pool.tile([128, PAIRS], f32)
        negsum = s_pool.tile([128, PAIRS], f32)

        # rhs rows round-robin across the two HWDGE queues (sync, scalar)
        # and the SWDGE queue (gpsimd); small per-pair tensors ride scalar.
        rhs_q = [nc.sync, nc.scalar, nc.gpsimd]

        # transfer issue is software-pipelined two pairs ahead of compute so
        # the scalar engine's exp work never gates posting of upcoming
        # descriptors to its DMA queue.
        tiles = {}

        def issue(p):
            b0, b1 = 2 * p, 2 * p + 1
            # each row's rhs arrives as two chunk-half transfers on different
            # queues: halves the per-pair arrival latency and keeps all three
            # DMA queues temporally aligned on the same pair window.
            r0 = rhs_pool.tile([128, CH, K * T], fp8, tag="rhs")
            rhs_q[(2 * b0) % 3].dma_start(r0[:, 0:2], rhs_d[b0, :, 0:2])
            rhs_q[(2 * b0 + 1) % 3].dma_start(r0[:, 2:4], rhs_d[b0, :, 2:4])
            r1 = rhs_pool.tile([128, CH, K * T], fp8, tag="rhs")
            rhs_q[(2 * b1) % 3].dma_start(r1[:, 0:2], rhs_d[b1, :, 0:2])
            rhs_q[(2 * b1 + 1) % 3].dma_start(r1[:, 2:4], rhs_d[b1, :, 2:4])
            lt0 = lhs_pool.tile([128, CH, T], fp8, tag="lhs")
            nc.scalar.dma_start(lt0[:], lhsT_d[b0])
            lt1 = lhs_pool.tile([128, CH, T], fp8, tag="lhs")
            nc.scalar.dma_start(lt1[:], lhsT_d[b1])
            c2t = c2_pool.tile([6, K * T], fp8, tag="c2")
            nc.scalar.dma_start(c2t[:], c2p_d[p])
            tiles[p] = (r0, r1, lt0, lt1, c2t)

        issue(0)
        issue(1)
        for p in range(PAIRS):
            if p + 2 < PAIRS:
                issue(p + 2)
            b0, b1 = 2 * p, 2 * p + 1
            r0, r1, lt0, lt1, c2t = tiles.pop(p)

            m = m_pool.tile([128, K], f32, tag="m")

            for h, (k0, kw) in enumerate(SLABS):
                w = kw * T
                ps = ps_pool.tile([128, w], f32, tag="ps")
                for c in range(CH):
                    for off, nw in SLAB_BLOCKS[h]:
                        nc.tensor.matmul(
                            ps[0:64, off:off + nw],
                            lt0[:, c, :],
                            r0[:, c, k0 * T + off:k0 * T + off + nw],
                            start=(c == 0), stop=False,
                            tile_position=(0, 0),
                        )
                        nc.tensor.matmul(
                            ps[64:128, off:off + nw],
                            lt1[:, c, :],
                            r1[:, c, k0 * T + off:k0 * T + off + nw],
                            start=(c == 0), stop=False,
                            tile_position=(0, 64),
                        )
                for off, nw in SLAB_BLOCKS[h]:
                    nc.tensor.matmul(
                        ps[:, off:off + nw],
                        sel[:],
                        c2t[:, k0 * T + off:k0 * T + off + nw],
                        start=False, stop=True,
                    )
                nc.vector.tensor_reduce(
                    out=m[:, k0:k0 + kw],
                    in_=ps[:].rearrange("p (k s) -> p k s", s=T),
                    op=mybir.AluOpType.min,
                    axis=mybir.AxisListType.X,
                )

            # clamp m into [q2n, q2n - ln eps] (per partition) so that
            # exp(-mc + q2n) == clamp(exp(-m + q2n), eps, 1) exactly; then the
            # scalar engine produces pos/neg sums via its accumulator.
            mc = e_pool.tile([128, K], f32, tag="e")
            nc.vector.tensor_scalar(
                out=mc[:],
                in0=m[:],
                scalar1=q2t[:, p:p + 1],
                scalar2=q2ht[:, p:p + 1],
                op0=mybir.AluOpType.max,
                op1=mybir.AluOpType.min,
            )
            ed = e_pool.tile([128, K], f32, tag="ed")
            nc.scalar.activation(
                ed[:, 0:NPOS],
                mc[:, 0:NPOS],
                mybir.ActivationFunctionType.Exp,
                bias=q2t[:, p:p + 1],
                scale=-1.0,
                accum_out=possum[:, p:p + 1],
            )
            nc.scalar.activation(
                ed[:, NPOS:K],
                mc[:, NPOS:K],
                mybir.ActivationFunctionType.Exp,
                bias=q2t[:, p:p + 1],
                scale=-1.0,
                accum_out=negsum[:, p:p + 1],
            )

        den = s_pool.tile([128, PAIRS], f32)
        nc.vector.tensor_add(den[:], possum[:], negsum[:])
        nc.vector.tensor_scalar_add(den[:], den[:], EPS)
        nc.vector.reciprocal(den[:], den[:])
        nc.vector.tensor_mul(den[:], den[:], possum[:])
        lnr = s_pool.tile([128, PAIRS], f32)
        nc.scalar.activation(lnr[:], den[:], mybir.ActivationFunctionType.Ln)
        nc.sync.dma_start(out_d[:], lnr[:])


def _ensure_axon_hooks():
    """bass_utils' trace path imports antenv.axon_hooks, which this image
    lacks; install a functional shim driving NTFF capture via libaxon."""
    try:
        import antenv.axon_hooks  # noqa: F401

        return
    except ImportError:
        pass
    import contextlib
    import ctypes
    import os
    import sys
    import types

    try:
        import antenv
    except ImportError:
        return
    mod = types.ModuleType("antenv.axon_hooks")
    _hook_box = [None]
    mod.set_axon_ntff_profile_hook = lambda h: _hook_box.__setitem__(0, h)
    mod.get_axon_ntff_profile_hook = lambda: _hook_box[0]
    sys.modules["antenv.axon_hooks"] = mod
    antenv.axon_hooks = mod

    so_path = "/opt/axon/libaxon_pjrt.so"
    if not os.path.exists(so_path):
        return
    try:
        lib = ctypes.CDLL(so_path)
        if not hasattr(lib, "axon_start_nrt_profile"):
            return
        lib.axon_start_nrt_profile.argtypes = [
            ctypes.POINTER(ctypes.c_int64),
            ctypes.c_size_t,
        ]
        lib.axon_start_nrt_profile.restype = ctypes.c_int64
        lib.axon_stop_nrt_profile.argtypes = [ctypes.c_char_p]
        lib.axon_stop_nrt_profile.restype = ctypes.c_int64

        @contextlib.contextmanager
        def _hook(output_dir, device_ids):
            import jax

            jax.devices()
            if device_ids:
                ids = (ctypes.c_int64 * len(device_ids))(*device_ids)
                rc = lib.axon_start_nrt_profile(ids, len(device_ids))
            else:
                rc = lib.axon_start_nrt_profile(None, 0)
            if rc != 0:
                raise RuntimeError(f"axon_start_nrt_profile rc={rc}")
            try:
                yield
            finally:
                n = lib.axon_stop_nrt_profile(str(output_dir).encode())
                print(f"profile: {n} file(s) written to {output_dir}", file=sys.stderr)

        mod.set_axon_ntff_profile_hook(_hook)
    except Exception:
        pass


def kernel(**inputs):
    global LAST_EXEC_NS, LAST_RESULTS
    import sys
    import time

    _ensure_axon_hooks()
    import concourse.bacc as bacc
    from concourse.bass_utils import run_bass_kernel_spmd

    t0 = time.time()
    in_maps = _prep(inputs)
    print(f"[v2] prep done {time.time()-t0:.1f}s", file=sys.stderr, flush=True)
    nc = bacc.Bacc("TRN2", target_bir_lowering=False, debug=False, num_devices=NCORES)
    _build(nc)
    nc.finalize()
    print(f"[v2] build done {time.time()-t0:.1f}s", file=sys.stderr, flush=True)
    res = run_bass_kernel_spmd(nc, in_maps, list(range(NCORES)))
    print(f"[v2] run done {time.time()-t0:.1f}s", file=sys.stderr, flush=True)
    LAST_EXEC_NS = res.exec_time_ns
    LAST_RESULTS = res

    total = 0.0
    for ci in range(NCORES):
        lnr = np.asarray(res.results[ci]["out"], dtype=np.float64)
        n = CORE_COUNTS[ci]
        for bl in range(n):
            pr, half = bl // 2, bl % 2
            total += lnr[half * 64:(half + 1) * 64, pr].sum()
    return np.float32(-500.0 * total / float(B))


# revision 6
# speedup vs baseline: 1.0861x; 1.0049x over previous
"""Trainium2 Bass kernel for nn_ContrastiveLoss (retrieval_knn).

Changes vs baseline kernel.py:
- all DMA on hardware-DGE queues (sync + scalar), none on gpsimd/SWDGE;
  rhs rows alternate between the two queues.
- consolidated matmuls: one matmul per (half, chunk, slab) -> 16 qc matmuls
  per pair instead of 56 (fewer LDWEIGHTS + pipeline drains).
- c2 folded via a K=6 fp8 selector matmul (hi/mid/lo planes) instead of the
  f32r K=4 matmul: same N but fp8 streams 2 cols/cycle.
"""

import numpy as np
import ml_dtypes

B = 222
NB = 444
T = 64
D = 512
V = 6
K = 54
NPOS = 2 * V
EPS = 1e-8
LN_EPS = float(np.log(np.float32(1e-8)))
NCORES = 8
BL = 28
PAIRS = BL // 2
CH = 4
SLABS = [(0, 32), (32, 22)]   # candidate slabs: 2048 + 1408 cols
SLAB_BLOCKS = {
    0: [(0, 512), (512, 512), (1024, 512), (1536, 512)],
    1: [(0, 512), (512, 512), (1024, 384)],
}
C0 = 512.0

CORE_STARTS = [0, 28, 56, 84, 112, 140, 168, 195]
CORE_COUNTS = [28, 28, 28, 28, 28, 28, 27, 27]

LAST_EXEC_NS = None
LAST_RESULTS = None


def _fp8_triple(x):
    hi = x.astype(ml_dtypes.float8_e4m3fn)
    r1 = x - hi.astype(np.float32)
    mid = r1.astype(ml_dtypes.float8_e4m3fn)
    lo = (r1 - mid.astype(np.float32)).astype(ml_dtypes.float8_e4m3fn)
    return hi, mid, lo


def _prep(inputs):
    emb = np.ascontiguousarray(np.asarray(inputs["embeddings"]), dtype=np.float32)
    ips = np.asarray(inputs["indices_posself"]).astype(np.int64)
    ipc = np.asarray(inputs["indices_poscross"]).astype(np.int64)
    ineg = np.asarray(inputs["indices_neg"]).astype(np.int64)
    osh = np.asarray(inputs["order_to_shuffle"]).astype(np.int64)
    pos = np.concatenate([ips, ipc], axis=1)
    combined = np.concatenate([pos, ineg, osh[pos]], axis=1)  # (222, 54)

    bank8 = emb.astype(ml_dtypes.float8_e4m3fn)
    bankf = bank8.astype(np.float32)
    c2 = np.einsum(
        "jsd,jsd->js", bankf.astype(np.float64), bankf.astype(np.float64)
    ).astype(np.float32)

    lhs_all = (-2.0 * bankf[:B]).astype(ml_dtypes.float8_e4m3fn)

    sel = np.zeros((6, 128), np.float32)
    sel[0:3, 0:64] = 1.0
    sel[3:6, 64:128] = 1.0
    sel8 = sel.astype(ml_dtypes.float8_e4m3fn)

    in_maps = []
    for ci in range(NCORES):
        s, n = CORE_STARTS[ci], CORE_COUNTS[ci]
        rows = np.array(list(range(s, s + n)) + [s] * (BL - n))
        cmb = combined[rows]

        g = bank8[cmb]                                        # (28,54,64,512) fp8
        rhs = np.ascontiguousarray(
            g.reshape(BL, K, T, CH, 128).transpose(0, 4, 3, 1, 2).reshape(BL, 128, CH, K * T)
        )
        lt = np.ascontiguousarray(
            lhs_all[rows].reshape(BL, T, CH, 128).transpose(0, 3, 2, 1)
        )                                                     # (28,128,4,64)

        c2g = c2[cmb].reshape(BL, K * T) - C0
        hi, mid, lo = _fp8_triple(c2g)
        c2p = np.empty((PAIRS, 6, K * T), ml_dtypes.float8_e4m3fn)
        c2p[:, 0] = hi[0::2]
        c2p[:, 1] = mid[0::2]
        c2p[:, 2] = lo[0::2]
        c2p[:, 3] = hi[1::2]
        c2p[:, 4] = mid[1::2]
        c2p[:, 5] = lo[1::2]

        q2row = c2[rows]
        q2n = np.empty((128, PAIRS), np.float32)
        for p in range(PAIRS):
            q2n[0:64, p] = -(q2row[2 * p] + C0)
            q2n[64:128, p] = -(q2row[2 * p + 1] + C0)
        q2h = (q2n - LN_EPS).astype(np.float32)
        in_maps.append(
            {"rhs": rhs, "lhsT": lt, "c2p": c2p, "q2n": q2n, "q2h": q2h, "sel": sel8}
        )
    return in_maps


def _build(nc):
    import concourse.tile as tile
    import concourse.mybir as mybir
    from contextlib import ExitStack

    dt = mybir.dt
    f32 = dt.float32
    fp8 = dt.float8e4

    rhs_d = nc.dram_tensor("rhs", [BL, 128, CH, K * T], fp8, kind="ExternalInput")
    lhsT_d = nc.dram_tensor("lhsT", [BL, 128, CH, T], fp8, kind="ExternalInput")
    c2p_d = nc.dram_tensor("c2p", [PAIRS, 6, K * T], fp8, kind="ExternalInput")
    q2_d = nc.dram_tensor("q2n", [128, PAIRS], f32, kind="ExternalInput")
    q2h_d = nc.dram_tensor("q2h", [128, PAIRS], f32, kind="ExternalInput")
    sel_d = nc.dram_tensor("sel", [6, 128], fp8, kind="ExternalInput")
    out_d = nc.dram_tensor("out", [128, PAIRS], f32, kind="ExternalOutput")

    with tile.TileContext(nc) as tc, ExitStack() as ctx:
        rhs_pool = ctx.enter_context(tc.tile_pool(name="rhs", bufs=12))
        lhs_pool = ctx.enter_context(tc.tile_pool(name="lhs", bufs=12))
        c2_pool = ctx.enter_context(tc.tile_pool(name="c2", bufs=7))
        ps_pool = ctx.enter_context(tc.tile_pool(name="ps", bufs=2, space="PSUM"))
        m_pool = ctx.enter_context(tc.tile_pool(name="m", bufs=3))
        e_pool = ctx.enter_context(tc.tile_pool(name="e", bufs=3))
        s_pool = ctx.enter_context(tc.tile_pool(name="s", bufs=1))

        sel = s_pool.tile([6, 128], fp8)
        nc.scalar.dma_start(sel[:], sel_d[:])
        q2t = s_pool.tile([128, PAIRS], f32)
        nc.scalar.dma_start(q2t[:], q2_d[:])
        q2ht = s_pool.tile([128, PAIRS], f32)
        nc.scalar.dma_start(q2ht[:], q2h_d[:])
        possum = s_pool.tile([128, PAIRS], f32)
        negsum = s_pool.tile([128, PAIRS], f32)

        # rhs rows round-robin across the two HWDGE queues (sync, scalar)
        # and the SWDGE queue (gpsimd); small per-pair tensors ride scalar.
        rhs_q = [nc.sync, nc.scalar, nc.gpsimd]

        # transfer issue is software-pipelined two pairs ahead of compute so
        # the scalar engine's exp work never gates posting of upcoming
        # descriptors to its DMA queue.
        tiles = {}

        def issue(p):
            b0, b1 = 2 * p, 2 * p + 1
            # each row's rhs arrives as two chunk-half transfers on different
            # queues: halves the per-pair arrival latency and keeps all three
            # DMA queues temporally aligned on the same pair window.
            r0 = rhs_pool.tile([128, CH, K * T], fp8, tag="rhs")
            rhs_q[(2 * b0) % 3].dma_start(r0[:, 0:2], rhs_d[b0, :, 0:2])
            rhs_q[(2 * b0 + 1) % 3].dma_start(r0[:, 2:4], rhs_d[b0, :, 2:4])
            r1 = rhs_pool.tile([128, CH, K * T], fp8, tag="rhs")
            rhs_q[(2 * b1) % 3].dma_start(r1[:, 0:2], rhs_d[b1, :, 0:2])
            rhs_q[(2 * b1 + 1) % 3].dma_start(r1[:, 2:4], rhs_d[b1, :, 2:4])
            lt0 = lhs_pool.tile([128, CH, T], fp8, tag="lhs")
            nc.scalar.dma_start(lt0[:], lhsT_d[b0])
            lt1 = lhs_pool.tile([128, CH, T], fp8, tag="lhs")
            nc.scalar.dma_start(lt1[:], lhsT_d[b1])
            c2t = c2_pool.tile([6, K * T], fp8, tag="c2")
            nc.scalar.dma_start(c2t[:], c2p_d[p])
            tiles[p] = (r0, r1, lt0, lt1, c2t)

        for p0 in range(4):
            issue(p0)
        for p in range(PAIRS):
            if p + 4 < PAIRS:
                issue(p + 4)
            b0, b1 = 2 * p, 2 * p + 1
            r0, r1, lt0, lt1, c2t = tiles.pop(p)

            m = m_pool.tile([128, K], f32, tag="m")

            for h, (k0, kw) in enumerate(SLABS):
                w = kw * T
                ps = ps_pool.tile([128, w], f32, tag="ps")
                for c in range(CH):
                    for off, nw in SLAB_BLOCKS[h]:
                        nc.tensor.matmul(
                            ps[0:64, off:off + nw],
                            lt0[:, c, :],
                            r0[:, c, k0 * T + off:k0 * T + off + nw],
                            start=(c == 0), stop=False,
                            tile_position=(0, 0),
                        )
                        nc.tensor.matmul(
                            ps[64:128, off:off + nw],
                            lt1[:, c, :],
                            r1[:, c, k0 * T + off:k0 * T + off + nw],
                            start=(c == 0), stop=False,
                            tile_position=(0, 64),
                        )
                for off, nw in SLAB_BLOCKS[h]:
                    nc.tensor.matmul(
                        ps[:, off:off + nw],
                        sel[:],
                        c2t[:, k0 * T + off:k0 * T + off + nw],
                        start=False, stop=True,
                    )
                nc.vector.tensor_reduce(
                    out=m[:, k0:k0 + kw],
                    in_=ps[:].rearrange("p (k s) -> p k s", s=T),
                    op=mybir.AluOpType.min,
                    axis=mybir.AxisListType.X,
                )

            # clamp m into [q2n, q2n - ln eps] (per partition) so that
            # exp(-mc + q2n) == clamp(exp(-m + q2n), eps, 1) exactly; then the
            # scalar engine produces pos/neg sums via its accumulator.
            mc = e_pool.tile([128, K], f32, tag="e")
            nc.vector.tensor_scalar(
                out=mc[:],
                in0=m[:],
                scalar1=q2t[:, p:p + 1],
                scalar2=q2ht[:, p:p + 1],
                op0=mybir.AluOpType.max,
                op1=mybir.AluOpType.min,
            )
            ed = e_pool.tile([128, K], f32, tag="ed")
            nc.scalar.activation(
                ed[:, 0:NPOS],
                mc[:, 0:NPOS],
                mybir.ActivationFunctionType.Exp,
                bias=q2t[:, p:p + 1],
                scale=-1.0,
                accum_out=possum[:, p:p + 1],
            )
            nc.scalar.activation(
                ed[:, NPOS:K],
                mc[:, NPOS:K],
                mybir.ActivationFunctionType.Exp,
                bias=q2t[:, p:p + 1],
                scale=-1.0,
                accum_out=negsum[:, p:p + 1],
            )

        den = s_pool.tile([128, PAIRS], f32)
        nc.vector.tensor_add(den[:], possum[:], negsum[:])
        nc.vector.tensor_scalar_add(den[:], den[:], EPS)
        nc.vector.reciprocal(den[:], den[:])
        nc.vector.tensor_mul(den[:], den[:], possum[:])
        lnr = s_pool.tile([128, PAIRS], f32)
        nc.scalar.activation(lnr[:], den[:], mybir.ActivationFunctionType.Ln)
        nc.sync.dma_start(out_d[:], lnr[:])


def _ensure_axon_hooks():
    """bass_utils' trace path imports antenv.axon_hooks, which this image
    lacks; install a functional shim driving NTFF capture via libaxon."""
    try:
        import antenv.axon_hooks  # noqa: F401

        return
    except ImportError:
        pass
    import contextlib
    import ctypes
    import os
    import sys
    import types

    try:
        import antenv
    except ImportError:
        return
    mod = types.ModuleType("antenv.axon_hooks")
    _hook_box = [None]
    mod.set_axon_ntff_profile_hook = lambda h: _hook_box.__setitem__(0, h)
    mod.get_axon_ntff_profile_hook = lambda: _hook_box[0]
    sys.modules["antenv.axon_hooks"] = mod
    antenv.axon_hooks = mod

    so_path = "/opt/axon/libaxon_pjrt.so"
    if not os.path.exists(so_path):
        return
    try:
        lib = ctypes.CDLL(so_path)
        if not hasattr(lib, "axon_start_nrt_profile"):
            return
        lib.axon_start_nrt_profile.argtypes = [
            ctypes.POINTER(ctypes.c_int64),
            ctypes.c_size_t,
        ]
        lib.axon_start_nrt_profile.restype = ctypes.c_int64
        lib.axon_stop_nrt_profile.argtypes = [ctypes.c_char_p]
        lib.axon_stop_nrt_profile.restype = ctypes.c_int64

        @contextlib.contextmanager
        def _hook(output_dir, device_ids):
            import jax

            jax.devices()
            if device_ids:
                ids = (ctypes.c_int64 * len(device_ids))(*device_ids)
                rc = lib.axon_start_nrt_profile(ids, len(device_ids))
            else:
                rc = lib.axon_start_nrt_profile(None, 0)
            if rc != 0:
                raise RuntimeError(f"axon_start_nrt_profile rc={rc}")
            try:
                yield
            finally:
                n = lib.axon_stop_nrt_profile(str(output_dir).encode())
                print(f"profile: {n} file(s) written to {output_dir}", file=sys.stderr)

        mod.set_axon_ntff_profile_hook(_hook)
    except Exception:
        pass


def kernel(**inputs):
    global LAST_EXEC_NS, LAST_RESULTS
    import sys
    import time

    _ensure_axon_hooks()
    import concourse.bacc as bacc
    from concourse.bass_utils import run_bass_kernel_spmd

    t0 = time.time()
    in_maps = _prep(inputs)
    print(f"[v2] prep done {time.time()-t0:.1f}s", file=sys.stderr, flush=True)
    nc = bacc.Bacc("TRN2", target_bir_lowering=False, debug=False, num_devices=NCORES)
    _build(nc)
    nc.finalize()
    print(f"[v2] build done {time.time()-t0:.1f}s", file=sys.stderr, flush=True)
    res = run_bass_kernel_spmd(nc, in_maps, list(range(NCORES)))
    print(f"[v2] run done {time.time()-t0:.1f}s", file=sys.stderr, flush=True)
    LAST_EXEC_NS = res.exec_time_ns
    LAST_RESULTS = res

    total = 0.0
    for ci in range(NCORES):
        lnr = np.asarray(res.results[ci]["out"], dtype=np.float64)
        n = CORE_COUNTS[ci]
        for bl in range(n):
            pr, half = bl // 2, bl % 2
            total += lnr[half * 64:(half + 1) * 64, pr].sum()
    return np.float32(-500.0 * total / float(B))


# revision 7
# speedup vs baseline: 1.0937x; 1.0070x over previous
"""Trainium2 Bass kernel for nn_ContrastiveLoss (retrieval_knn).

Changes vs baseline kernel.py:
- all DMA on hardware-DGE queues (sync + scalar), none on gpsimd/SWDGE;
  rhs rows alternate between the two queues.
- consolidated matmuls: one matmul per (half, chunk, slab) -> 16 qc matmuls
  per pair instead of 56 (fewer LDWEIGHTS + pipeline drains).
- c2 folded via a K=6 fp8 selector matmul (hi/mid/lo planes) instead of the
  f32r K=4 matmul: same N but fp8 streams 2 cols/cycle.
"""

import numpy as np
import ml_dtypes

B = 222
NB = 444
T = 64
D = 512
V = 6
K = 54
NPOS = 2 * V
EPS = 1e-8
LN_EPS = float(np.log(np.float32(1e-8)))
NCORES = 8
BL = 28
PAIRS = BL // 2
CH = 4
# balanced slabs: each DVE slab reduce hides inside the opposite PE slab
SLABS = [(0, 27), (27, 27)]   # candidate slabs: 1728 + 1728 cols
SLAB_BLOCKS = {
    0: [(0, 512), (512, 512), (1024, 512), (1536, 192)],
    1: [(0, 512), (512, 512), (1024, 512), (1536, 192)],
}
C0 = 512.0

CORE_STARTS = [0, 28, 56, 84, 112, 140, 168, 195]
CORE_COUNTS = [28, 28, 28, 28, 28, 28, 27, 27]

LAST_EXEC_NS = None
LAST_RESULTS = None


def _fp8_triple(x):
    hi = x.astype(ml_dtypes.float8_e4m3fn)
    r1 = x - hi.astype(np.float32)
    mid = r1.astype(ml_dtypes.float8_e4m3fn)
    lo = (r1 - mid.astype(np.float32)).astype(ml_dtypes.float8_e4m3fn)
    return hi, mid, lo


def _prep(inputs):
    emb = np.ascontiguousarray(np.asarray(inputs["embeddings"]), dtype=np.float32)
    ips = np.asarray(inputs["indices_posself"]).astype(np.int64)
    ipc = np.asarray(inputs["indices_poscross"]).astype(np.int64)
    ineg = np.asarray(inputs["indices_neg"]).astype(np.int64)
    osh = np.asarray(inputs["order_to_shuffle"]).astype(np.int64)
    pos = np.concatenate([ips, ipc], axis=1)
    combined = np.concatenate([pos, ineg, osh[pos]], axis=1)  # (222, 54)

    bank8 = emb.astype(ml_dtypes.float8_e4m3fn)
    bankf = bank8.astype(np.float32)
    c2 = np.einsum(
        "jsd,jsd->js", bankf.astype(np.float64), bankf.astype(np.float64)
    ).astype(np.float32)

    lhs_all = (-2.0 * bankf[:B]).astype(ml_dtypes.float8_e4m3fn)

    sel = np.zeros((6, 128), np.float32)
    sel[0:3, 0:64] = 1.0
    sel[3:6, 64:128] = 1.0
    sel8 = sel.astype(ml_dtypes.float8_e4m3fn)

    in_maps = []
    for ci in range(NCORES):
        s, n = CORE_STARTS[ci], CORE_COUNTS[ci]
        rows = np.array(list(range(s, s + n)) + [s] * (BL - n))
        cmb = combined[rows]

        g = bank8[cmb]                                        # (28,54,64,512) fp8
        rhs = np.ascontiguousarray(
            g.reshape(BL, K, T, CH, 128).transpose(0, 4, 3, 1, 2).reshape(BL, 128, CH, K * T)
        )
        lt = np.ascontiguousarray(
            lhs_all[rows].reshape(BL, T, CH, 128).transpose(0, 3, 2, 1)
        )                                                     # (28,128,4,64)

        c2g = c2[cmb].reshape(BL, K * T) - C0
        hi, mid, lo = _fp8_triple(c2g)
        c2p = np.empty((PAIRS, 6, K * T), ml_dtypes.float8_e4m3fn)
        c2p[:, 0] = hi[0::2]
        c2p[:, 1] = mid[0::2]
        c2p[:, 2] = lo[0::2]
        c2p[:, 3] = hi[1::2]
        c2p[:, 4] = mid[1::2]
        c2p[:, 5] = lo[1::2]

        q2row = c2[rows]
        q2n = np.empty((128, PAIRS), np.float32)
        for p in range(PAIRS):
            q2n[0:64, p] = -(q2row[2 * p] + C0)
            q2n[64:128, p] = -(q2row[2 * p + 1] + C0)
        q2h = (q2n - LN_EPS).astype(np.float32)
        in_maps.append(
            {"rhs": rhs, "lhsT": lt, "c2p": c2p, "q2n": q2n, "q2h": q2h, "sel": sel8}
        )
    return in_maps


def _build(nc):
    import concourse.tile as tile
    import concourse.mybir as mybir
    from contextlib import ExitStack

    dt = mybir.dt
    f32 = dt.float32
    fp8 = dt.float8e4

    rhs_d = nc.dram_tensor("rhs", [BL, 128, CH, K * T], fp8, kind="ExternalInput")
    lhsT_d = nc.dram_tensor("lhsT", [BL, 128, CH, T], fp8, kind="ExternalInput")
    c2p_d = nc.dram_tensor("c2p", [PAIRS, 6, K * T], fp8, kind="ExternalInput")
    q2_d = nc.dram_tensor("q2n", [128, PAIRS], f32, kind="ExternalInput")
    q2h_d = nc.dram_tensor("q2h", [128, PAIRS], f32, kind="ExternalInput")
    sel_d = nc.dram_tensor("sel", [6, 128], fp8, kind="ExternalInput")
    out_d = nc.dram_tensor("out", [128, PAIRS], f32, kind="ExternalOutput")

    with tile.TileContext(nc) as tc, ExitStack() as ctx:
        rhs_pool = ctx.enter_context(tc.tile_pool(name="rhs", bufs=12))
        lhs_pool = ctx.enter_context(tc.tile_pool(name="lhs", bufs=12))
        c2_pool = ctx.enter_context(tc.tile_pool(name="c2", bufs=7))
        ps_pool = ctx.enter_context(tc.tile_pool(name="ps", bufs=2, space="PSUM"))
        m_pool = ctx.enter_context(tc.tile_pool(name="m", bufs=3))
        e_pool = ctx.enter_context(tc.tile_pool(name="e", bufs=3))
        s_pool = ctx.enter_context(tc.tile_pool(name="s", bufs=1))

        sel = s_pool.tile([6, 128], fp8)
        nc.scalar.dma_start(sel[:], sel_d[:])
        q2t = s_pool.tile([128, PAIRS], f32)
        nc.scalar.dma_start(q2t[:], q2_d[:])
        q2ht = s_pool.tile([128, PAIRS], f32)
        nc.scalar.dma_start(q2ht[:], q2h_d[:])
        possum = s_pool.tile([128, PAIRS], f32)
        negsum = s_pool.tile([128, PAIRS], f32)

        # rhs rows round-robin across the two HWDGE queues (sync, scalar)
        # and the SWDGE queue (gpsimd); small per-pair tensors ride scalar.
        rhs_q = [nc.sync, nc.scalar, nc.gpsimd]

        # transfer issue is software-pipelined two pairs ahead of compute so
        # the scalar engine's exp work never gates posting of upcoming
        # descriptors to its DMA queue.
        tiles = {}

        def issue(p):
            b0, b1 = 2 * p, 2 * p + 1
            # each row's rhs arrives as two chunk-half transfers on different
            # queues: halves the per-pair arrival latency and keeps all three
            # DMA queues temporally aligned on the same pair window.
            r0 = rhs_pool.tile([128, CH, K * T], fp8, tag="rhs")
            rhs_q[(2 * b0) % 3].dma_start(r0[:, 0:2], rhs_d[b0, :, 0:2])
            rhs_q[(2 * b0 + 1) % 3].dma_start(r0[:, 2:4], rhs_d[b0, :, 2:4])
            r1 = rhs_pool.tile([128, CH, K * T], fp8, tag="rhs")
            rhs_q[(2 * b1) % 3].dma_start(r1[:, 0:2], rhs_d[b1, :, 0:2])
            rhs_q[(2 * b1 + 1) % 3].dma_start(r1[:, 2:4], rhs_d[b1, :, 2:4])
            lt0 = lhs_pool.tile([128, CH, T], fp8, tag="lhs")
            nc.scalar.dma_start(lt0[:], lhsT_d[b0])
            lt1 = lhs_pool.tile([128, CH, T], fp8, tag="lhs")
            nc.scalar.dma_start(lt1[:], lhsT_d[b1])
            c2t = c2_pool.tile([6, K * T], fp8, tag="c2")
            nc.scalar.dma_start(c2t[:], c2p_d[p])
            tiles[p] = (r0, r1, lt0, lt1, c2t)

        for p0 in range(4):
            issue(p0)
        for p in range(PAIRS):
            if p + 4 < PAIRS:
                issue(p + 4)
            b0, b1 = 2 * p, 2 * p + 1
            r0, r1, lt0, lt1, c2t = tiles.pop(p)

            m = m_pool.tile([128, K], f32, tag="m")

            for h, (k0, kw) in enumerate(SLABS):
                w = kw * T
                ps = ps_pool.tile([128, w], f32, tag="ps")
                for c in range(CH):
                    for off, nw in SLAB_BLOCKS[h]:
                        nc.tensor.matmul(
                            ps[0:64, off:off + nw],
                            lt0[:, c, :],
                            r0[:, c, k0 * T + off:k0 * T + off + nw],
                            start=(c == 0), stop=False,
                            tile_position=(0, 0),
                        )
                        nc.tensor.matmul(
                            ps[64:128, off:off + nw],
                            lt1[:, c, :],
                            r1[:, c, k0 * T + off:k0 * T + off + nw],
                            start=(c == 0), stop=False,
                            tile_position=(0, 64),
                        )
                for off, nw in SLAB_BLOCKS[h]:
                    nc.tensor.matmul(
                        ps[:, off:off + nw],
                        sel[:],
                        c2t[:, k0 * T + off:k0 * T + off + nw],
                        start=False, stop=True,
                    )
                nc.vector.tensor_reduce(
                    out=m[:, k0:k0 + kw],
                    in_=ps[:].rearrange("p (k s) -> p k s", s=T),
                    op=mybir.AluOpType.min,
                    axis=mybir.AxisListType.X,
                )

            # clamp m into [q2n, q2n - ln eps] (per partition) so that
            # exp(-mc + q2n) == clamp(exp(-m + q2n), eps, 1) exactly; then the
            # scalar engine produces pos/neg sums via its accumulator.
            mc = e_pool.tile([128, K], f32, tag="e")
            nc.vector.tensor_scalar(
                out=mc[:],
                in0=m[:],
                scalar1=q2t[:, p:p + 1],
                scalar2=q2ht[:, p:p + 1],
                op0=mybir.AluOpType.max,
                op1=mybir.AluOpType.min,
            )
            ed = e_pool.tile([128, K], f32, tag="ed")
            nc.scalar.activation(
                ed[:, 0:NPOS],
                mc[:, 0:NPOS],
                mybir.ActivationFunctionType.Exp,
                bias=q2t[:, p:p + 1],
                scale=-1.0,
                accum_out=possum[:, p:p + 1],
            )
            nc.scalar.activation(
                ed[:, NPOS:K],
                mc[:, NPOS:K],
                mybir.ActivationFunctionType.Exp,
                bias=q2t[:, p:p + 1],
                scale=-1.0,
                accum_out=negsum[:, p:p + 1],
            )

        den = s_pool.tile([128, PAIRS], f32)
        nc.vector.tensor_add(den[:], possum[:], negsum[:])
        nc.vector.tensor_scalar_add(den[:], den[:], EPS)
        nc.vector.reciprocal(den[:], den[:])
        nc.vector.tensor_mul(den[:], den[:], possum[:])
        lnr = s_pool.tile([128, PAIRS], f32)
        nc.scalar.activation(lnr[:], den[:], mybir.ActivationFunctionType.Ln)
        nc.sync.dma_start(out_d[:], lnr[:])


def _ensure_axon_hooks():
    """bass_utils' trace path imports antenv.axon_hooks, which this image
    lacks; install a functional shim driving NTFF capture via libaxon."""
    try:
        import antenv.axon_hooks  # noqa: F401

        return
    except ImportError:
        pass
    import contextlib
    import ctypes
    import os
    import sys
    import types

    try:
        import antenv
    except ImportError:
        return
    mod = types.ModuleType("antenv.axon_hooks")
    _hook_box = [None]
    mod.set_axon_ntff_profile_hook = lambda h: _hook_box.__setitem__(0, h)
    mod.get_axon_ntff_profile_hook = lambda: _hook_box[0]
    sys.modules["antenv.axon_hooks"] = mod
    antenv.axon_hooks = mod

    so_path = "/opt/axon/libaxon_pjrt.so"
    if not os.path.exists(so_path):
        return
    try:
        lib = ctypes.CDLL(so_path)
        if not hasattr(lib, "axon_start_nrt_profile"):
            return
        lib.axon_start_nrt_profile.argtypes = [
            ctypes.POINTER(ctypes.c_int64),
            ctypes.c_size_t,
        ]
        lib.axon_start_nrt_profile.restype = ctypes.c_int64
        lib.axon_stop_nrt_profile.argtypes = [ctypes.c_char_p]
        lib.axon_stop_nrt_profile.restype = ctypes.c_int64

        @contextlib.contextmanager
        def _hook(output_dir, device_ids):
            import jax

            jax.devices()
            if device_ids:
                ids = (ctypes.c_int64 * len(device_ids))(*device_ids)
                rc = lib.axon_start_nrt_profile(ids, len(device_ids))
            else:
                rc = lib.axon_start_nrt_profile(None, 0)
            if rc != 0:
                raise RuntimeError(f"axon_start_nrt_profile rc={rc}")
            try:
                yield
            finally:
                n = lib.axon_stop_nrt_profile(str(output_dir).encode())
                print(f"profile: {n} file(s) written to {output_dir}", file=sys.stderr)

        mod.set_axon_ntff_profile_hook(_hook)
    except Exception:
        pass


def kernel(**inputs):
    global LAST_EXEC_NS, LAST_RESULTS
    import sys
    import time

    _ensure_axon_hooks()
    import concourse.bacc as bacc
    from concourse.bass_utils import run_bass_kernel_spmd

    t0 = time.time()
    in_maps = _prep(inputs)
    print(f"[v2] prep done {time.time()-t0:.1f}s", file=sys.stderr, flush=True)
    nc = bacc.Bacc("TRN2", target_bir_lowering=False, debug=False, num_devices=NCORES)
    _build(nc)
    nc.finalize()
    print(f"[v2] build done {time.time()-t0:.1f}s", file=sys.stderr, flush=True)
    res = run_bass_kernel_spmd(nc, in_maps, list(range(NCORES)))
    print(f"[v2] run done {time.time()-t0:.1f}s", file=sys.stderr, flush=True)
    LAST_EXEC_NS = res.exec_time_ns
    LAST_RESULTS = res

    total = 0.0
    for ci in range(NCORES):
        lnr = np.asarray(res.results[ci]["out"], dtype=np.float64)
        n = CORE_COUNTS[ci]
        for bl in range(n):
            pr, half = bl // 2, bl % 2
            total += lnr[half * 64:(half + 1) * 64, pr].sum()
    return np.float32(-500.0 * total / float(B))
